# revision 8
# baseline (speedup 1.0000x reference)
"""Multi-head attention (B=2, S=2048, D=1024, H=16) on 8 Trainium2 NeuronCores.

Sharding: core c -> batch b = c // 4, head-group g = c % 4 (4 heads = 256 proj
dims per core). Each core computes its 4 heads' attention plus the matching
slice of the output projection; the host sums the 4 partial outputs per batch
and adds bo.

Device layouts (matmul operands float32r = fp32 bits at bf16 PE rate):
  qT/kT [o, s]   : proj from host-transposed Q/K (contraction on partitions)
  v     [s, o]   : natural layout + ones column per head (softmax denominator
                   rides along row 64 of the PV matmul output)
  scoresT [k, q] : head pairs row-packed on the PE (base_partition 0/64);
                   both halves of a [128,1024] PSUM tile -> one wide Exp
  outT  [d, q]   : unnormalized; moved off PSUM fast, normalized with
                   reciprocal_approx_fast + GpSimd partition_broadcast
  out_pT [o, q]  : local slice of x @ Wo.T; host transposes + sums + bias
"""

import ml_dtypes
import numpy as np

import concourse.bass as bass
import concourse.mybir as mybir
import concourse.tile as tile
from concourse import bacc
from concourse.bass_utils import run_bass_kernel_spmd

B, S, D, H = 2, 2048, 1024, 16
OL = 256          # local projection dims (4 heads x 64)
NI = D // 128     # contraction chunks for projections
NK = S // 128     # key chunks
NQ = S // 512     # query blocks

_CACHE = {}


def _build():
    DT = mybir.dt.float16
    F32 = mybir.dt.float32
    AF = mybir.ActivationFunctionType

    nc = bacc.Bacc("TRN2", target_bir_lowering=False, debug=False, num_devices=8)

    qt_d = nc.dram_tensor("qt", [D, S], DT, kind="ExternalInput").ap() \
        .rearrange("(c p) s -> c p s", p=128)
    kt_d = nc.dram_tensor("kt", [D, S], DT, kind="ExternalInput").ap() \
        .rearrange("(c p) s -> c p s", p=128)
    vt_d = nc.dram_tensor("vt", [D, S], DT, kind="ExternalInput").ap() \
        .rearrange("(c p) s -> c p s", p=128)
    wq_d = nc.dram_tensor("wqt", [D, OL], DT, kind="ExternalInput").ap() \
        .rearrange("(c p) o -> c p o", p=128)
    wk_d = nc.dram_tensor("wkt", [D, OL], DT, kind="ExternalInput").ap() \
        .rearrange("(c p) o -> c p o", p=128)
    wv_d = nc.dram_tensor("wvt", [D, OL], DT, kind="ExternalInput").ap() \
        .rearrange("(c p) o -> c p o", p=128)
    bq_d = nc.dram_tensor("bq2", [2, 128, 1], F32, kind="ExternalInput").ap()
    bk_d = nc.dram_tensor("bk2", [2, 128, 1], F32, kind="ExternalInput").ap()
    bv_d = nc.dram_tensor("bv1", [1, OL], DT, kind="ExternalInput").ap()
    wo_d = nc.dram_tensor("wot", [OL, D], DT, kind="ExternalInput").ap() \
        .rearrange("(c p) o -> c p o", p=128)
    out_d = nc.dram_tensor("out_t", [D, S], F32, kind="ExternalOutput").ap() \
        .rearrange("(c p) s -> c p s", p=128)

    with tile.TileContext(nc) as tc:
        with (
            tc.tile_pool(name="per", bufs=1) as per,
            tc.tile_pool(name="wp", bufs=1) as wp,
            tc.tile_pool(name="ip", bufs=1) as ip,
            tc.tile_pool(name="pr", bufs=3) as pr,
            tc.tile_pool(name="sm", bufs=3) as sm,
            tc.tile_pool(name="ot", bufs=2) as ot,
            tc.tile_pool(name="osg", bufs=3) as osg,
            tc.tile_pool(name="pj", bufs=2, space="PSUM") as pj,
            tc.tile_pool(name="p1", bufs=2, space="PSUM") as p1,
            tc.tile_pool(name="px", bufs=2, space="PSUM") as px,
        ):
            # --- persistent tiles
            qt_sb = [per.tile([128, S], DT, tag=f"qt{m}", name=f"qt{m}")
                     for m in range(2)]
            kt_sb = [per.tile([128, S], DT, tag=f"kt{m}", name=f"kt{m}")
                     for m in range(2)]
            v_sb = [per.tile([128, 4, 65], DT, tag=f"v{sc}", name=f"v{sc}")
                    for sc in range(NK)]
            wo_sb = [per.tile([128, D], DT, tag=f"wo{c}", name=f"wo{c}")
                     for c in range(2)]
            bq_sb = [per.tile([128, 1], F32, tag=f"bq{m}", name=f"bq{m}")
                     for m in range(2)]
            bk_sb = [per.tile([128, 1], F32, tag=f"bk{m}", name=f"bk{m}")
                     for m in range(2)]
            bv_sb = per.tile([1, OL], DT, tag="bv", name="bv")
            ones_f = per.tile([1, 128], F32, tag="ones_f", name="ones_f")
            vones_f = per.tile([128, 1], F32, tag="vones_f", name="vones_f")
            ones_r = per.tile([1, 128], DT, tag="ones_r", name="ones_r")
            nc.vector.memset(ones_f[:], 1.0)
            nc.vector.memset(vones_f[:], 1.0)
            nc.vector.tensor_copy(ones_r[:], ones_f[:])

            for m in range(2):
                nc.sync.dma_start(bq_sb[m][:], bq_d[m])
                nc.sync.dma_start(bk_sb[m][:], bk_d[m])
            nc.sync.dma_start(bv_sb[:], bv_d)
            for c in range(2):
                nc.sync.dma_start(wo_sb[c][:], wo_d[c])

            def load_wa(w_dr, a_dr):
                ws, as_ = [], []
                for i in range(NI):
                    w = wp.tile([128, OL], DT, tag=f"w{i}", name=f"w{i}")
                    nc.sync.dma_start(w[:], w_dr[i])
                    ws.append(w)
                    a = ip.tile([128, S], DT, tag=f"a{i}", name=f"a{i}")
                    nc.sync.dma_start(a[:], a_dr[i])
                    as_.append(a)
                return ws, as_

            def proj_qk(w_dr, a_dr, bias_sb, dst_sb):
                # dst[o, s] = sum_i W[o, i] X[s, i]; i-streamed, 2 psum accs
                ws, as_ = load_wa(w_dr, a_dr)
                for m in range(2):
                    for s in range(4):
                        acc = pj.tile([128, 512], F32, tag="pj", name="pj")
                        for i in range(NI):
                            nc.tensor.matmul(
                                acc[:],
                                ws[i][:, m * 128:(m + 1) * 128],
                                as_[i][:, s * 512:(s + 1) * 512],
                                start=(i == 0),
                                stop=(i == NI - 1),
                            )
                        nc.vector.tensor_scalar_add(
                            dst_sb[m][:, s * 512:(s + 1) * 512],
                            acc[:], bias_sb[m][:],
                        )

            def proj_v(w_dr, a_dr):
                # v[s, o] = sum_i X[s, i] W[o, i] + bv (bias via K=1 matmul)
                ws, as_ = load_wa(w_dr, a_dr)
                for sc in range(NK):
                    acc = pj.tile([128, OL], F32, tag="pj", name="pj")
                    for i in range(NI):
                        nc.tensor.matmul(
                            acc[:],
                            as_[i][:, sc * 128:(sc + 1) * 128],
                            ws[i][:],
                            start=(i == 0),
                            stop=False,
                        )
                    nc.tensor.matmul(
                        acc[:], ones_r[:], bv_sb[:],
                        start=False, stop=True,
                    )
                    for h in range(4):
                        nc.vector.tensor_copy(
                            v_sb[sc][:, h, 0:64],
                            acc[:, h * 64:(h + 1) * 64],
                        )
                    nc.vector.tensor_copy(
                        v_sb[sc][:, :, 64:65],
                        vones_f[:].to_broadcast((128, 4, 1)),
                    )

            proj_qk(wk_d, kt_d, bk_sb, kt_sb)
            proj_v(wv_d, vt_d)
            proj_qk(wq_d, qt_d, bq_sb, qt_sb)

            # --- attention + output projection, per query block
            # OP of qb-1 is spread into qb's pair-0 kc loop (PE slack there);
            # PV matmuls trail the exp by 2 kc steps so the PE never waits.
            def emit_op(qb, ots_prev):
                ops = []
                for oc in range(8):
                    osl = slice(oc * 128, (oc + 1) * 128)
                    pso = p1.tile([128, 512], F32, tag="s", name="pso")
                    for c in range(2):
                        nc.tensor.matmul(
                            pso[:], wo_sb[c][:, osl], ots_prev[c][:],
                            start=(c == 0), stop=(c == 1),
                        )
                    st = osg.tile([128, 512], F32, tag="st", name="st")
                    nc.vector.tensor_copy(st[:], pso[:])
                    nc.sync.dma_start(
                        out_d[oc][:, qb * 512:(qb + 1) * 512], st[:])
                    ops.append(None)

            ots_prev = None
            for qb in range(NQ):
                qsl = slice(qb * 512, (qb + 1) * 512)
                ots = [ot.tile([128, 512], DT, tag=f"c{c}", name=f"otc{c}")
                       for c in range(2)]
                for pair in range(2):
                    acc = [px.tile([65, 512], F32, tag="x", name="acc")
                           for _ in range(2)]
                    pend = []
                    op_iter = None
                    if pair == 0 and ots_prev is not None:
                        op_iter = iter(range(8))
                    for kc in range(NK):
                        ksl = slice(kc * 128, (kc + 1) * 128)
                        ps1 = p1.tile([128, 1024], F32, tag="s", name="s")
                        for hh in range(2):
                            psl = slice(hh * 64, (hh + 1) * 64)
                            nc.tensor.matmul(
                                ps1[:, hh * 512:(hh + 1) * 512],
                                kt_sb[pair][psl, ksl],
                                qt_sb[pair][psl, qsl],
                                start=True, stop=True,
                            )
                        prob = pr.tile([128, 1024], DT, tag="p", name="p")
                        nc.scalar.activation(
                            prob[:], ps1[:], AF.Exp, scale=0.125
                        )
                        pend.append((kc, prob))
                        if len(pend) > 2:
                            pkc, pprob = pend.pop(0)
                            for hh in range(2):
                                nc.tensor.matmul(
                                    acc[hh][:], v_sb[pkc][:, pair * 2 + hh, :],
                                    pprob[:, hh * 512:(hh + 1) * 512],
                                    start=(pkc == 0), stop=(pkc == NK - 1),
                                )
                        if op_iter is not None and kc % 2 == 1:
                            oc = next(op_iter, None)
                            if oc is not None:
                                osl = slice(oc * 128, (oc + 1) * 128)
                                pso = p1.tile([128, 512], F32, tag="s",
                                              name="pso")
                                for c in range(2):
                                    nc.tensor.matmul(
                                        pso[:], wo_sb[c][:, osl],
                                        ots_prev[c][:],
                                        start=(c == 0), stop=(c == 1),
                                    )
                                st = osg.tile([128, 512], F32, tag="st",
                                              name="st")
                                nc.vector.tensor_copy(st[:], pso[:])
                                nc.sync.dma_start(
                                    out_d[oc][:, (qb - 1) * 512:qb * 512],
                                    st[:])
                    for pkc, pprob in pend:
                        for hh in range(2):
                            nc.tensor.matmul(
                                acc[hh][:], v_sb[pkc][:, pair * 2 + hh, :],
                                pprob[:, hh * 512:(hh + 1) * 512],
                                start=(pkc == 0), stop=(pkc == NK - 1),
                            )
                    # normalize off-bank: free both acc banks first
                    uns, dens = [], []
                    for hh in range(2):
                        un = sm.tile([64, 512], F32, tag=f"un{hh}",
                                     name=f"un{hh}")
                        nc.vector.tensor_copy(un[:], acc[hh][0:64, :])
                        den = sm.tile([1, 512], F32, tag=f"den{hh}",
                                      name=f"den{hh}")
                        nc.vector.tensor_copy(den[:], acc[hh][64:65, :])
                        uns.append(un)
                        dens.append(den)
                    for hh in range(2):
                        rec = sm.tile([1, 512], F32, tag="rec", name="rec")
                        nc.vector.reciprocal_approx_fast(rec[:], dens[hh][:])
                        rb = sm.tile([64, 512], F32, tag="rb", name="rb")
                        nc.gpsimd.partition_broadcast(rb[:], rec[:])
                        nc.vector.tensor_mul(
                            ots[pair][hh * 64:(hh + 1) * 64, :],
                            uns[hh][:], rb[:],
                        )
                ots_prev = ots
            emit_op(NQ - 1, ots_prev)

    nc.compile()
    return nc


def _get_nc():
    if "nc" not in _CACHE:
        _CACHE["nc"] = _build()
    return _CACHE["nc"]


def kernel(Q, K, V, Wq, bq, Wk, bk, Wv, bv, Wo, bo):
    nc = _get_nc()
    f = np.float32
    bf = np.float16
    in_maps = []
    for core in range(8):
        b, g = divmod(core, 4)
        sl = slice(g * OL, (g + 1) * OL)
        in_maps.append({
            "qt": np.ascontiguousarray(Q[b].T, dtype=bf),
            "kt": np.ascontiguousarray(K[b].T, dtype=bf),
            "vt": np.ascontiguousarray(V[b].T, dtype=bf),
            "wqt": np.ascontiguousarray(Wq[sl].T, dtype=bf),
            "wkt": np.ascontiguousarray(Wk[sl].T, dtype=bf),
            "wvt": np.ascontiguousarray(Wv[sl].T, dtype=bf),
            "bq2": np.ascontiguousarray(bq[sl].reshape(2, 128, 1), dtype=f),
            "bk2": np.ascontiguousarray(bk[sl].reshape(2, 128, 1), dtype=f),
            "bv1": np.ascontiguousarray(bv[sl].reshape(1, OL), dtype=bf),
            "wot": np.ascontiguousarray(Wo[:, sl].T, dtype=bf),
        })
    res = run_bass_kernel_spmd(nc, in_maps, core_ids=list(range(8)))
    out = np.empty((B, S, D), np.float32)
    for b in range(B):
        acc = res.results[b * 4 + 0]["out_t"].astype(np.float64)
        for g in range(1, 4):
            acc += res.results[b * 4 + g]["out_t"]
        out[b] = (acc.T + bo).astype(np.float32)
    return out


# revision 9
# speedup vs baseline: 1.0752x; 1.0752x over previous
"""Multi-head attention (B=2, S=2048, D=1024, H=16) on 8 Trainium2 NeuronCores.

Sharding: core c -> batch b = c // 4, head-group g = c % 4 (4 heads = 256 proj
dims per core). Each core computes its 4 heads' attention plus the matching
slice of the output projection; the host sums the 4 partial outputs per batch
and adds bo.

Device layouts (matmul operands float32r = fp32 bits at bf16 PE rate):
  qT/kT [o, s]   : proj from host-transposed Q/K (contraction on partitions)
  v     [s, o]   : natural layout + ones column per head (softmax denominator
                   rides along row 64 of the PV matmul output)
  scoresT [k, q] : head pairs row-packed on the PE (base_partition 0/64);
                   both halves of a [128,1024] PSUM tile -> one wide Exp
  outT  [d, q]   : unnormalized; moved off PSUM fast, normalized with
                   reciprocal_approx_fast + GpSimd partition_broadcast
  out_pT [o, q]  : local slice of x @ Wo.T; host transposes + sums + bias
"""

import ml_dtypes
import numpy as np

import concourse.bass as bass
import concourse.mybir as mybir
import concourse.tile as tile
from concourse import bacc
from concourse.bass_utils import run_bass_kernel_spmd

B, S, D, H = 2, 2048, 1024, 16
OL = 256          # local projection dims (4 heads x 64)
NI = D // 128     # contraction chunks for projections
NK = S // 128     # key chunks
NQ = S // 512     # query blocks

_CACHE = {}


def _build():
    DT = mybir.dt.float16
    F32 = mybir.dt.float32
    AF = mybir.ActivationFunctionType

    nc = bacc.Bacc("TRN2", target_bir_lowering=False, debug=False, num_devices=8)

    qt_d = nc.dram_tensor("qt", [D, S], DT, kind="ExternalInput").ap() \
        .rearrange("(c p) s -> c p s", p=128)
    kt_d = nc.dram_tensor("kt", [D, S], DT, kind="ExternalInput").ap() \
        .rearrange("(c p) s -> c p s", p=128)
    vt_d = nc.dram_tensor("vt", [D, S], DT, kind="ExternalInput").ap() \
        .rearrange("(c p) s -> c p s", p=128)
    wq_d = nc.dram_tensor("wqt", [D, OL], DT, kind="ExternalInput").ap() \
        .rearrange("(c p) o -> c p o", p=128)
    wk_d = nc.dram_tensor("wkt", [D, OL], DT, kind="ExternalInput").ap() \
        .rearrange("(c p) o -> c p o", p=128)
    wv_d = nc.dram_tensor("wvt", [D, OL], DT, kind="ExternalInput").ap() \
        .rearrange("(c p) o -> c p o", p=128)
    bq_d = nc.dram_tensor("bq2", [2, 128, 1], F32, kind="ExternalInput").ap()
    bk_d = nc.dram_tensor("bk2", [2, 128, 1], F32, kind="ExternalInput").ap()
    bv_d = nc.dram_tensor("bv1", [1, OL], DT, kind="ExternalInput").ap()
    wo_d = nc.dram_tensor("wot", [OL, D], DT, kind="ExternalInput").ap() \
        .rearrange("(c p) o -> c p o", p=128)
    out_d = nc.dram_tensor("out_t", [D, S], F32, kind="ExternalOutput").ap() \
        .rearrange("(c p) s -> c p s", p=128)

    with tile.TileContext(nc) as tc:
        with (
            tc.tile_pool(name="per", bufs=1) as per,
            tc.tile_pool(name="wp", bufs=1) as wp,
            tc.tile_pool(name="ip", bufs=1) as ip,
            tc.tile_pool(name="pr", bufs=8) as pr,
            tc.tile_pool(name="sm", bufs=3) as sm,
            tc.tile_pool(name="ot", bufs=2) as ot,
            tc.tile_pool(name="osg", bufs=3) as osg,
            tc.tile_pool(name="pj", bufs=2, space="PSUM") as pj,
            tc.tile_pool(name="p1", bufs=2, space="PSUM") as p1,
            tc.tile_pool(name="px", bufs=2, space="PSUM") as px,
        ):
            # --- persistent tiles
            qt_sb = [per.tile([128, S], DT, tag=f"qt{m}", name=f"qt{m}")
                     for m in range(2)]
            kt_sb = [per.tile([128, S], DT, tag=f"kt{m}", name=f"kt{m}")
                     for m in range(2)]
            v_sb = [per.tile([128, 4, 65], DT, tag=f"v{sc}", name=f"v{sc}")
                    for sc in range(NK)]
            wo_sb = [per.tile([128, D], DT, tag=f"wo{c}", name=f"wo{c}")
                     for c in range(2)]
            bq_sb = [per.tile([128, 1], F32, tag=f"bq{m}", name=f"bq{m}")
                     for m in range(2)]
            bk_sb = [per.tile([128, 1], F32, tag=f"bk{m}", name=f"bk{m}")
                     for m in range(2)]
            bv_sb = per.tile([1, OL], DT, tag="bv", name="bv")
            ones_f = per.tile([1, 128], F32, tag="ones_f", name="ones_f")
            vones_f = per.tile([128, 1], F32, tag="vones_f", name="vones_f")
            ones_r = per.tile([1, 128], DT, tag="ones_r", name="ones_r")
            nc.vector.memset(ones_f[:], 1.0)
            nc.vector.memset(vones_f[:], 1.0)
            nc.vector.tensor_copy(ones_r[:], ones_f[:])

            for m in range(2):
                nc.sync.dma_start(bq_sb[m][:], bq_d[m])
                nc.sync.dma_start(bk_sb[m][:], bk_d[m])
            nc.sync.dma_start(bv_sb[:], bv_d)
            for c in range(2):
                nc.sync.dma_start(wo_sb[c][:], wo_d[c])

            def load_wa(w_dr, a_dr):
                ws, as_ = [], []
                for i in range(NI):
                    w = wp.tile([128, OL], DT, tag=f"w{i}", name=f"w{i}")
                    nc.sync.dma_start(w[:], w_dr[i])
                    ws.append(w)
                    a = ip.tile([128, S], DT, tag=f"a{i}", name=f"a{i}")
                    nc.sync.dma_start(a[:], a_dr[i])
                    as_.append(a)
                return ws, as_

            def proj_qk(w_dr, a_dr, bias_sb, dst_sb):
                # dst[o, s] = sum_i W[o, i] X[s, i]; i-streamed, 2 psum accs
                ws, as_ = load_wa(w_dr, a_dr)
                for m in range(2):
                    for s in range(4):
                        acc = pj.tile([128, 512], F32, tag="pj", name="pj")
                        for i in range(NI):
                            nc.tensor.matmul(
                                acc[:],
                                ws[i][:, m * 128:(m + 1) * 128],
                                as_[i][:, s * 512:(s + 1) * 512],
                                start=(i == 0),
                                stop=(i == NI - 1),
                            )
                        nc.scalar.activation(
                            dst_sb[m][:, s * 512:(s + 1) * 512],
                            acc[:], AF.Identity, bias=bias_sb[m][:],
                        )

            def v_chain(vws, vas, sc):
                # one v[s, o] output chunk: 8-deep accumulation + bias matmul
                acc = pj.tile([128, OL], F32, tag="pj", name="pj")
                for i in range(NI):
                    nc.tensor.matmul(
                        acc[:],
                        vas[i][:, sc * 128:(sc + 1) * 128],
                        vws[i][:],
                        start=(i == 0),
                        stop=False,
                    )
                nc.tensor.matmul(
                    acc[:], ones_r[:], bv_sb[:], start=False, stop=True
                )
                for h in range(4):
                    nc.vector.tensor_copy(
                        v_sb[sc][:, h, 0:64],
                        acc[:, h * 64:(h + 1) * 64],
                    )

            for sc in range(NK):
                nc.vector.tensor_copy(
                    v_sb[sc][:, :, 64:65],
                    vones_f[:].to_broadcast((128, 4, 1)),
                )
            proj_qk(wk_d, kt_d, bk_sb, kt_sb)
            proj_qk(wq_d, qt_d, bq_sb, qt_sb)
            vws, vas = load_wa(wv_d, vt_d)

            # --- attention + output projection, per query block
            # OP of qb-1 is spread into qb's pair-0 kc loop (PE slack there);
            # PV matmuls trail the exp by 2 kc steps so the PE never waits.
            def emit_op(qb, ots_prev):
                ops = []
                for oc in range(8):
                    osl = slice(oc * 128, (oc + 1) * 128)
                    pso = pj.tile([128, 512], F32, tag="pj", name="pso")
                    for c in range(2):
                        nc.tensor.matmul(
                            pso[:], wo_sb[c][:, osl], ots_prev[c][:],
                            start=(c == 0), stop=(c == 1),
                        )
                    st = osg.tile([128, 512], F32, tag="st", name="st")
                    nc.vector.tensor_copy(st[:], pso[:])
                    nc.sync.dma_start(
                        out_d[oc][:, qb * 512:(qb + 1) * 512], st[:])
                    ops.append(None)

            ots_prev = None
            for qb in range(NQ):
                qsl = slice(qb * 512, (qb + 1) * 512)
                ots = [ot.tile([128, 512], DT, tag=f"c{c}", name=f"otc{c}")
                       for c in range(2)]
                for pair in range(2):
                    first_blk = (qb == 0 and pair == 0)
                    depth = 6 if first_blk else 2
                    acc = [px.tile([65, 512], F32, tag="x", name="acc")
                           for _ in range(2)]
                    pend = []
                    op_iter = None
                    if pair == 0 and ots_prev is not None:
                        op_iter = iter(range(8))
                    for kc in range(NK):
                        if first_blk:
                            v_chain(vws, vas, kc)
                        ksl = slice(kc * 128, (kc + 1) * 128)
                        ps1 = p1.tile([128, 1024], F32, tag="s", name="s")
                        for hh in range(2):
                            psl = slice(hh * 64, (hh + 1) * 64)
                            nc.tensor.matmul(
                                ps1[:, hh * 512:(hh + 1) * 512],
                                kt_sb[pair][psl, ksl],
                                qt_sb[pair][psl, qsl],
                                start=True, stop=True,
                            )
                        prob = pr.tile([128, 1024], DT, tag="p", name="p")
                        nc.scalar.activation(
                            prob[:], ps1[:], AF.Exp, scale=0.125
                        )
                        pend.append((kc, prob))
                        if len(pend) > depth:
                            pkc, pprob = pend.pop(0)
                            for hh in range(2):
                                nc.tensor.matmul(
                                    acc[hh][:], v_sb[pkc][:, pair * 2 + hh, :],
                                    pprob[:, hh * 512:(hh + 1) * 512],
                                    start=(pkc == 0), stop=(pkc == NK - 1),
                                )
                        if op_iter is not None and kc % 2 == 1:
                            oc = next(op_iter, None)
                            if oc is not None:
                                osl = slice(oc * 128, (oc + 1) * 128)
                                pso = pj.tile([128, 512], F32, tag="pj",
                                              name="pso")
                                for c in range(2):
                                    nc.tensor.matmul(
                                        pso[:], wo_sb[c][:, osl],
                                        ots_prev[c][:],
                                        start=(c == 0), stop=(c == 1),
                                    )
                                st = osg.tile([128, 512], F32, tag="st",
                                              name="st")
                                nc.vector.tensor_copy(st[:], pso[:])
                                nc.sync.dma_start(
                                    out_d[oc][:, (qb - 1) * 512:qb * 512],
                                    st[:])
                    for pkc, pprob in pend:
                        for hh in range(2):
                            nc.tensor.matmul(
                                acc[hh][:], v_sb[pkc][:, pair * 2 + hh, :],
                                pprob[:, hh * 512:(hh + 1) * 512],
                                start=(pkc == 0), stop=(pkc == NK - 1),
                            )
                    # normalize off-bank: free both acc banks first
                    uns, dens = [], []
                    for hh in range(2):
                        un = sm.tile([64, 512], F32, tag=f"un{hh}",
                                     name=f"un{hh}")
                        nc.vector.tensor_copy(un[:], acc[hh][0:64, :])
                        den = sm.tile([1, 512], F32, tag=f"den{hh}",
                                      name=f"den{hh}")
                        nc.vector.tensor_copy(den[:], acc[hh][64:65, :])
                        uns.append(un)
                        dens.append(den)
                    for hh in range(2):
                        rec = sm.tile([1, 512], F32, tag="rec", name="rec")
                        nc.vector.reciprocal_approx_fast(rec[:], dens[hh][:])
                        rb = sm.tile([64, 512], F32, tag="rb", name="rb")
                        nc.gpsimd.partition_broadcast(rb[:], rec[:])
                        nc.vector.tensor_mul(
                            ots[pair][hh * 64:(hh + 1) * 64, :],
                            uns[hh][:], rb[:],
                        )
                ots_prev = ots
            emit_op(NQ - 1, ots_prev)

    nc.compile()
    return nc


def _get_nc():
    if "nc" not in _CACHE:
        _CACHE["nc"] = _build()
    return _CACHE["nc"]


def kernel(Q, K, V, Wq, bq, Wk, bk, Wv, bv, Wo, bo):
    nc = _get_nc()
    f = np.float32
    bf = np.float16
    in_maps = []
    for core in range(8):
        b, g = divmod(core, 4)
        sl = slice(g * OL, (g + 1) * OL)
        in_maps.append({
            "qt": np.ascontiguousarray(Q[b].T, dtype=bf),
            "kt": np.ascontiguousarray(K[b].T, dtype=bf),
            "vt": np.ascontiguousarray(V[b].T, dtype=bf),
            "wqt": np.ascontiguousarray(Wq[sl].T, dtype=bf),
            "wkt": np.ascontiguousarray(Wk[sl].T, dtype=bf),
            "wvt": np.ascontiguousarray(Wv[sl].T, dtype=bf),
            "bq2": np.ascontiguousarray(bq[sl].reshape(2, 128, 1), dtype=f),
            "bk2": np.ascontiguousarray(bk[sl].reshape(2, 128, 1), dtype=f),
            "bv1": np.ascontiguousarray(bv[sl].reshape(1, OL), dtype=bf),
            "wot": np.ascontiguousarray(Wo[:, sl].T, dtype=bf),
        })
    res = run_bass_kernel_spmd(nc, in_maps, core_ids=list(range(8)))
    out = np.empty((B, S, D), np.float32)
    for b in range(B):
        acc = res.results[b * 4 + 0]["out_t"].astype(np.float64)
        for g in range(1, 4):
            acc += res.results[b * 4 + g]["out_t"]
        out[b] = (acc.T + bo).astype(np.float32)
    return out


# revision 10
# speedup vs baseline: 1.1024x; 1.0253x over previous
"""Multi-head attention (B=2, S=2048, D=1024, H=16) on 8 Trainium2 NeuronCores.

Sharding: core c -> batch b = c // 4, head-group g = c % 4 (4 heads = 256 proj
dims per core). Each core computes its 4 heads' attention plus the matching
slice of the output projection; the host sums the 4 partial outputs per batch
and adds bo.

Device layouts (matmul operands float32r = fp32 bits at bf16 PE rate):
  qT/kT [o, s]   : proj from host-transposed Q/K (contraction on partitions)
  v     [s, o]   : natural layout + ones column per head (softmax denominator
                   rides along row 64 of the PV matmul output)
  scoresT [k, q] : head pairs row-packed on the PE (base_partition 0/64);
                   both halves of a [128,1024] PSUM tile -> one wide Exp
  outT  [d, q]   : unnormalized; moved off PSUM fast, normalized with
                   reciprocal_approx_fast + GpSimd partition_broadcast
  out_pT [o, q]  : local slice of x @ Wo.T; host transposes + sums + bias
"""

import ml_dtypes
import numpy as np

import concourse.bass as bass
import concourse.mybir as mybir
import concourse.tile as tile
from concourse import bacc
from concourse.bass_utils import run_bass_kernel_spmd

B, S, D, H = 2, 2048, 1024, 16
OL = 256          # local projection dims (4 heads x 64)
NI = D // 128     # contraction chunks for projections
NK = S // 128     # key chunks
NQ = S // 512     # query blocks

_CACHE = {}


def _build():
    DT = mybir.dt.float16
    F32 = mybir.dt.float32
    AF = mybir.ActivationFunctionType

    nc = bacc.Bacc("TRN2", target_bir_lowering=False, debug=False, num_devices=8)

    qt_d = nc.dram_tensor("qt", [D, S], DT, kind="ExternalInput").ap() \
        .rearrange("(c p) s -> c p s", p=128)
    kt_d = nc.dram_tensor("kt", [D, S], DT, kind="ExternalInput").ap() \
        .rearrange("(c p) s -> c p s", p=128)
    vt_d = nc.dram_tensor("vt", [D, S], DT, kind="ExternalInput").ap() \
        .rearrange("(c p) s -> c p s", p=128)
    wq_d = nc.dram_tensor("wqt", [D, OL], DT, kind="ExternalInput").ap() \
        .rearrange("(c p) o -> c p o", p=128)
    wk_d = nc.dram_tensor("wkt", [D, OL], DT, kind="ExternalInput").ap() \
        .rearrange("(c p) o -> c p o", p=128)
    wv_d = nc.dram_tensor("wvt", [D, OL], DT, kind="ExternalInput").ap() \
        .rearrange("(c p) o -> c p o", p=128)
    bq_d = nc.dram_tensor("bq2", [2, 128, 1], F32, kind="ExternalInput").ap()
    bk_d = nc.dram_tensor("bk2", [2, 128, 1], F32, kind="ExternalInput").ap()
    bv_d = nc.dram_tensor("bv1", [1, OL], DT, kind="ExternalInput").ap()
    wo_d = nc.dram_tensor("wot", [OL, D], DT, kind="ExternalInput").ap() \
        .rearrange("(c p) o -> c p o", p=128)
    out_d = nc.dram_tensor("out_t", [D, S], F32, kind="ExternalOutput").ap() \
        .rearrange("(c p) s -> c p s", p=128)

    with tile.TileContext(nc) as tc:
        with (
            tc.tile_pool(name="per", bufs=1) as per,
            tc.tile_pool(name="wp", bufs=1) as wp,
            tc.tile_pool(name="ip", bufs=1) as ip,
            tc.tile_pool(name="pr", bufs=8) as pr,
            tc.tile_pool(name="sm", bufs=3) as sm,
            tc.tile_pool(name="ot", bufs=2) as ot,
            tc.tile_pool(name="osg", bufs=3) as osg,
            tc.tile_pool(name="pj", bufs=2, space="PSUM") as pj,
            tc.tile_pool(name="p1", bufs=2, space="PSUM") as p1,
            tc.tile_pool(name="px", bufs=2, space="PSUM") as px,
        ):
            # --- persistent tiles
            qt_sb = [per.tile([128, S], DT, tag=f"qt{m}", name=f"qt{m}")
                     for m in range(2)]
            kt_sb = [per.tile([128, S], DT, tag=f"kt{m}", name=f"kt{m}")
                     for m in range(2)]
            v_sb = [per.tile([128, 4, 65], DT, tag=f"v{sc}", name=f"v{sc}")
                    for sc in range(NK)]
            wo_sb = [per.tile([128, D], DT, tag=f"wo{c}", name=f"wo{c}")
                     for c in range(2)]
            bq_sb = [per.tile([128, 1], F32, tag=f"bq{m}", name=f"bq{m}")
                     for m in range(2)]
            bk_sb = [per.tile([128, 1], F32, tag=f"bk{m}", name=f"bk{m}")
                     for m in range(2)]
            bv_sb = per.tile([1, OL], DT, tag="bv", name="bv")
            ones_f = per.tile([1, 128], F32, tag="ones_f", name="ones_f")
            vones_f = per.tile([128, 1], F32, tag="vones_f", name="vones_f")
            ones_r = per.tile([1, 128], DT, tag="ones_r", name="ones_r")
            nc.vector.memset(ones_f[:], 1.0)
            nc.vector.memset(vones_f[:], 1.0)
            nc.vector.tensor_copy(ones_r[:], ones_f[:])

            for m in range(2):
                nc.sync.dma_start(bq_sb[m][:], bq_d[m])
                nc.sync.dma_start(bk_sb[m][:], bk_d[m])
            nc.sync.dma_start(bv_sb[:], bv_d)
            for c in range(2):
                nc.sync.dma_start(wo_sb[c][:], wo_d[c])

            def load_wa(w_dr, a_dr):
                # column-chunked activation DMA: the s-major projection
                # chains start as soon as their 512-col slices land
                ws, as_ = [], []
                for i in range(NI):
                    w = wp.tile([128, OL], DT, tag=f"w{i}", name=f"w{i}")
                    nc.sync.dma_start(w[:], w_dr[i])
                    ws.append(w)
                    as_.append(ip.tile([128, S], DT, tag=f"a{i}",
                                       name=f"a{i}"))
                for s4 in range(4):
                    csl = slice(s4 * 512, (s4 + 1) * 512)
                    for i in range(NI):
                        nc.sync.dma_start(as_[i][:, csl], a_dr[i][:, csl])
                return ws, as_

            def proj_qk(w_dr, a_dr, bias_sb, dst_sb):
                # dst[o, s] = sum_i W[o, i] X[s, i]; i-streamed, 2 psum accs
                ws, as_ = load_wa(w_dr, a_dr)
                for s in range(4):
                    for m in range(2):
                        acc = pj.tile([128, 512], F32, tag="pj", name="pj")
                        for i in range(NI):
                            nc.tensor.matmul(
                                acc[:],
                                ws[i][:, m * 128:(m + 1) * 128],
                                as_[i][:, s * 512:(s + 1) * 512],
                                start=(i == 0),
                                stop=(i == NI - 1),
                            )
                        nc.scalar.activation(
                            dst_sb[m][:, s * 512:(s + 1) * 512],
                            acc[:], AF.Identity, bias=bias_sb[m][:],
                        )

            def v_chain(vws, vas, sc):
                # one v[s, o] output chunk: 8-deep accumulation + bias matmul
                acc = pj.tile([128, OL], F32, tag="pj", name="pj")
                for i in range(NI):
                    nc.tensor.matmul(
                        acc[:],
                        vas[i][:, sc * 128:(sc + 1) * 128],
                        vws[i][:],
                        start=(i == 0),
                        stop=False,
                    )
                nc.tensor.matmul(
                    acc[:], ones_r[:], bv_sb[:], start=False, stop=True
                )
                for h in range(4):
                    nc.vector.tensor_copy(
                        v_sb[sc][:, h, 0:64],
                        acc[:, h * 64:(h + 1) * 64],
                    )

            for sc in range(NK):
                nc.vector.tensor_copy(
                    v_sb[sc][:, :, 64:65],
                    vones_f[:].to_broadcast((128, 4, 1)),
                )
            proj_qk(wk_d, kt_d, bk_sb, kt_sb)
            proj_qk(wq_d, qt_d, bq_sb, qt_sb)
            vws, vas = load_wa(wv_d, vt_d)

            # --- attention + output projection, per query block
            # OP of qb-1 is spread into qb's pair-0 kc loop (PE slack there);
            # PV matmuls trail the exp by 2 kc steps so the PE never waits.
            def emit_op(qb, ots_prev):
                ops = []
                for oc in range(8):
                    osl = slice(oc * 128, (oc + 1) * 128)
                    pso = pj.tile([128, 512], F32, tag="pj", name="pso")
                    for c in range(2):
                        nc.tensor.matmul(
                            pso[:], wo_sb[c][:, osl], ots_prev[c][:],
                            start=(c == 0), stop=(c == 1),
                        )
                    st = osg.tile([128, 512], F32, tag="st", name="st")
                    nc.vector.tensor_copy(st[:], pso[:])
                    nc.sync.dma_start(
                        out_d[oc][:, qb * 512:(qb + 1) * 512], st[:])
                    ops.append(None)

            ots_prev = None
            for qb in range(NQ):
                qsl = slice(qb * 512, (qb + 1) * 512)
                ots = [ot.tile([128, 512], DT, tag=f"c{c}", name=f"otc{c}")
                       for c in range(2)]
                for pair in range(2):
                    first_blk = (qb == 0 and pair == 0)
                    depth = 6 if first_blk else 2
                    acc = [px.tile([65, 512], F32, tag="x", name="acc")
                           for _ in range(2)]
                    pend = []
                    op_iter = None
                    if pair == 0 and ots_prev is not None:
                        op_iter = iter(range(8))
                    for kc in range(NK):
                        if first_blk:
                            v_chain(vws, vas, kc)
                        ksl = slice(kc * 128, (kc + 1) * 128)
                        ps1 = p1.tile([128, 1024], F32, tag="s", name="s")
                        for hh in range(2):
                            psl = slice(hh * 64, (hh + 1) * 64)
                            nc.tensor.matmul(
                                ps1[:, hh * 512:(hh + 1) * 512],
                                kt_sb[pair][psl, ksl],
                                qt_sb[pair][psl, qsl],
                                start=True, stop=True,
                            )
                        prob = pr.tile([128, 1024], DT, tag="p", name="p")
                        nc.scalar.activation(
                            prob[:], ps1[:], AF.Exp, scale=0.125
                        )
                        pend.append((kc, prob))
                        if len(pend) > depth:
                            pkc, pprob = pend.pop(0)
                            for hh in range(2):
                                nc.tensor.matmul(
                                    acc[hh][:], v_sb[pkc][:, pair * 2 + hh, :],
                                    pprob[:, hh * 512:(hh + 1) * 512],
                                    start=(pkc == 0), stop=(pkc == NK - 1),
                                )
                        if op_iter is not None and kc % 2 == 1:
                            oc = next(op_iter, None)
                            if oc is not None:
                                osl = slice(oc * 128, (oc + 1) * 128)
                                pso = pj.tile([128, 512], F32, tag="pj",
                                              name="pso")
                                for c in range(2):
                                    nc.tensor.matmul(
                                        pso[:], wo_sb[c][:, osl],
                                        ots_prev[c][:],
                                        start=(c == 0), stop=(c == 1),
                                    )
                                st = osg.tile([128, 512], F32, tag="st",
                                              name="st")
                                nc.vector.tensor_copy(st[:], pso[:])
                                nc.sync.dma_start(
                                    out_d[oc][:, (qb - 1) * 512:qb * 512],
                                    st[:])
                    for pkc, pprob in pend:
                        for hh in range(2):
                            nc.tensor.matmul(
                                acc[hh][:], v_sb[pkc][:, pair * 2 + hh, :],
                                pprob[:, hh * 512:(hh + 1) * 512],
                                start=(pkc == 0), stop=(pkc == NK - 1),
                            )
                    # normalize off-bank: free both acc banks first
                    uns, dens = [], []
                    for hh in range(2):
                        un = sm.tile([64, 512], F32, tag=f"un{hh}",
                                     name=f"un{hh}")
                        nc.vector.tensor_copy(un[:], acc[hh][0:64, :])
                        den = sm.tile([1, 512], F32, tag=f"den{hh}",
                                      name=f"den{hh}")
                        nc.vector.tensor_copy(den[:], acc[hh][64:65, :])
                        uns.append(un)
                        dens.append(den)
                    for hh in range(2):
                        rec = sm.tile([1, 512], F32, tag="rec", name="rec")
                        nc.vector.reciprocal_approx_fast(rec[:], dens[hh][:])
                        rb = sm.tile([64, 512], F32, tag="rb", name="rb")
                        nc.gpsimd.partition_broadcast(rb[:], rec[:])
                        nc.vector.tensor_mul(
                            ots[pair][hh * 64:(hh + 1) * 64, :],
                            uns[hh][:], rb[:],
                        )
                ots_prev = ots
            emit_op(NQ - 1, ots_prev)

    nc.compile()
    return nc


def _get_nc():
    if "nc" not in _CACHE:
        _CACHE["nc"] = _build()
    return _CACHE["nc"]


def kernel(Q, K, V, Wq, bq, Wk, bk, Wv, bv, Wo, bo):
    nc = _get_nc()
    f = np.float32
    bf = np.float16
    in_maps = []
    for core in range(8):
        b, g = divmod(core, 4)
        sl = slice(g * OL, (g + 1) * OL)
        in_maps.append({
            "qt": np.ascontiguousarray(Q[b].T, dtype=bf),
            "kt": np.ascontiguousarray(K[b].T, dtype=bf),
            "vt": np.ascontiguousarray(V[b].T, dtype=bf),
            "wqt": np.ascontiguousarray(Wq[sl].T, dtype=bf),
            "wkt": np.ascontiguousarray(Wk[sl].T, dtype=bf),
            "wvt": np.ascontiguousarray(Wv[sl].T, dtype=bf),
            "bq2": np.ascontiguousarray(bq[sl].reshape(2, 128, 1), dtype=f),
            "bk2": np.ascontiguousarray(bk[sl].reshape(2, 128, 1), dtype=f),
            "bv1": np.ascontiguousarray(bv[sl].reshape(1, OL), dtype=bf),
            "wot": np.ascontiguousarray(Wo[:, sl].T, dtype=bf),
        })
    res = run_bass_kernel_spmd(nc, in_maps, core_ids=list(range(8)))
    out = np.empty((B, S, D), np.float32)
    for b in range(B):
        acc = res.results[b * 4 + 0]["out_t"].astype(np.float64)
        for g in range(1, 4):
            acc += res.results[b * 4 + g]["out_t"]
        out[b] = (acc.T + bo).astype(np.float32)
    return out


# revision 11
# speedup vs baseline: 1.1084x; 1.0054x over previous
"""Multi-head attention (B=2, S=2048, D=1024, H=16) on 8 Trainium2 NeuronCores.

Sharding: core c -> batch b = c // 4, head-group g = c % 4 (4 heads = 256 proj
dims per core). Each core computes its 4 heads' attention plus the matching
slice of the output projection; the host sums the 4 partial outputs per batch
and adds bo.

Device layouts (matmul operands float32r = fp32 bits at bf16 PE rate):
  qT/kT [o, s]   : proj from host-transposed Q/K (contraction on partitions)
  v     [s, o]   : natural layout + ones column per head (softmax denominator
                   rides along row 64 of the PV matmul output)
  scoresT [k, q] : head pairs row-packed on the PE (base_partition 0/64);
                   both halves of a [128,1024] PSUM tile -> one wide Exp
  outT  [d, q]   : unnormalized; moved off PSUM fast, normalized with
                   reciprocal_approx_fast + GpSimd partition_broadcast
  out_pT [o, q]  : local slice of x @ Wo.T; host transposes + sums + bias
"""

import ml_dtypes
import numpy as np

import concourse.bass as bass
import concourse.mybir as mybir
import concourse.tile as tile
from concourse import bacc
from concourse.bass_utils import run_bass_kernel_spmd

B, S, D, H = 2, 2048, 1024, 16
OL = 256          # local projection dims (4 heads x 64)
NI = D // 128     # contraction chunks for projections
NK = S // 128     # key chunks
NQ = S // 512     # query blocks

_CACHE = {}


def _build():
    DT = mybir.dt.float16
    F32 = mybir.dt.float32
    AF = mybir.ActivationFunctionType

    nc = bacc.Bacc("TRN2", target_bir_lowering=False, debug=False, num_devices=8)

    qt_d = nc.dram_tensor("qt", [D, S], DT, kind="ExternalInput").ap() \
        .rearrange("(c p) s -> c p s", p=128)
    kt_d = nc.dram_tensor("kt", [D, S], DT, kind="ExternalInput").ap() \
        .rearrange("(c p) s -> c p s", p=128)
    vt_d = nc.dram_tensor("vt", [D, S], DT, kind="ExternalInput").ap() \
        .rearrange("(c p) s -> c p s", p=128)
    wq_d = nc.dram_tensor("wqt", [D, OL], DT, kind="ExternalInput").ap() \
        .rearrange("(c p) o -> c p o", p=128)
    wk_d = nc.dram_tensor("wkt", [D, OL], DT, kind="ExternalInput").ap() \
        .rearrange("(c p) o -> c p o", p=128)
    wv_d = nc.dram_tensor("wvt", [D, OL], DT, kind="ExternalInput").ap() \
        .rearrange("(c p) o -> c p o", p=128)
    bq_d = nc.dram_tensor("bq2", [2, 128, 1], F32, kind="ExternalInput").ap()
    bk_d = nc.dram_tensor("bk2", [2, 128, 1], F32, kind="ExternalInput").ap()
    bv_d = nc.dram_tensor("bv1", [1, OL], DT, kind="ExternalInput").ap()
    wo_d = nc.dram_tensor("wot", [OL, D], DT, kind="ExternalInput").ap() \
        .rearrange("(c p) o -> c p o", p=128)
    out_d = nc.dram_tensor("out_t", [D, S], F32, kind="ExternalOutput").ap() \
        .rearrange("(c p) s -> c p s", p=128)

    with tile.TileContext(nc) as tc:
        with (
            tc.tile_pool(name="per", bufs=1) as per,
            tc.tile_pool(name="wp", bufs=1) as wp,
            tc.tile_pool(name="ip", bufs=1) as ip,
            tc.tile_pool(name="pr", bufs=8) as pr,
            tc.tile_pool(name="sm", bufs=3) as sm,
            tc.tile_pool(name="ot", bufs=2) as ot,
            tc.tile_pool(name="osg", bufs=3) as osg,
            tc.tile_pool(name="pj", bufs=2, space="PSUM") as pj,
            tc.tile_pool(name="p1", bufs=2, space="PSUM") as p1,
            tc.tile_pool(name="px", bufs=2, space="PSUM") as px,
        ):
            # --- persistent tiles
            qt_sb = [per.tile([128, S], DT, tag=f"qt{m}", name=f"qt{m}")
                     for m in range(2)]
            kt_sb = [per.tile([128, S], DT, tag=f"kt{m}", name=f"kt{m}")
                     for m in range(2)]
            v_sb = [per.tile([128, 4, 65], DT, tag=f"v{sc}", name=f"v{sc}")
                    for sc in range(NK)]
            wo_sb = [per.tile([128, D], DT, tag=f"wo{c}", name=f"wo{c}")
                     for c in range(2)]
            bq_sb = [per.tile([128, 1], F32, tag=f"bq{m}", name=f"bq{m}")
                     for m in range(2)]
            bk_sb = [per.tile([128, 1], F32, tag=f"bk{m}", name=f"bk{m}")
                     for m in range(2)]
            bv_sb = per.tile([1, OL], DT, tag="bv", name="bv")
            ones_f = per.tile([1, 128], F32, tag="ones_f", name="ones_f")
            vones_f = per.tile([128, 1], F32, tag="vones_f", name="vones_f")
            ones_r = per.tile([1, 128], DT, tag="ones_r", name="ones_r")
            nc.vector.memset(ones_f[:], 1.0)
            nc.vector.memset(vones_f[:], 1.0)
            nc.vector.tensor_copy(ones_r[:], ones_f[:])

            for m in range(2):
                nc.sync.dma_start(bq_sb[m][:], bq_d[m])
                nc.sync.dma_start(bk_sb[m][:], bk_d[m])
            nc.sync.dma_start(bv_sb[:], bv_d)
            for c in range(2):
                nc.sync.dma_start(wo_sb[c][:], wo_d[c])

            def load_wa(w_dr, a_dr):
                # column-chunked activation DMA: the s-major projection
                # chains start as soon as their 512-col slices land
                ws, as_ = [], []
                for i in range(NI):
                    w = wp.tile([128, OL], DT, tag=f"w{i}", name=f"w{i}")
                    nc.sync.dma_start(w[:], w_dr[i])
                    ws.append(w)
                    as_.append(ip.tile([128, S], DT, tag=f"a{i}",
                                       name=f"a{i}"))
                for s4 in range(4):
                    csl = slice(s4 * 512, (s4 + 1) * 512)
                    for i in range(NI):
                        eng = nc.sync if i % 2 == 0 else nc.gpsimd
                        eng.dma_start(as_[i][:, csl], a_dr[i][:, csl])
                return ws, as_

            def proj_qk(w_dr, a_dr, bias_sb, dst_sb):
                # dst[o, s] = sum_i W[o, i] X[s, i]; i-streamed, 2 psum accs
                ws, as_ = load_wa(w_dr, a_dr)
                for s in range(4):
                    for m in range(2):
                        acc = pj.tile([128, 512], F32, tag="pj", name="pj")
                        for i in range(NI):
                            nc.tensor.matmul(
                                acc[:],
                                ws[i][:, m * 128:(m + 1) * 128],
                                as_[i][:, s * 512:(s + 1) * 512],
                                start=(i == 0),
                                stop=(i == NI - 1),
                            )
                        nc.scalar.activation(
                            dst_sb[m][:, s * 512:(s + 1) * 512],
                            acc[:], AF.Identity, bias=bias_sb[m][:],
                        )

            def v_chain(vws, vas, sc):
                # one v[s, o] output chunk: 8-deep accumulation + bias matmul
                acc = pj.tile([128, OL], F32, tag="pj", name="pj")
                for i in range(NI):
                    nc.tensor.matmul(
                        acc[:],
                        vas[i][:, sc * 128:(sc + 1) * 128],
                        vws[i][:],
                        start=(i == 0),
                        stop=False,
                    )
                nc.tensor.matmul(
                    acc[:], ones_r[:], bv_sb[:], start=False, stop=True
                )
                for h in range(4):
                    nc.vector.tensor_copy(
                        v_sb[sc][:, h, 0:64],
                        acc[:, h * 64:(h + 1) * 64],
                    )

            for sc in range(NK):
                nc.vector.tensor_copy(
                    v_sb[sc][:, :, 64:65],
                    vones_f[:].to_broadcast((128, 4, 1)),
                )
            proj_qk(wk_d, kt_d, bk_sb, kt_sb)
            proj_qk(wq_d, qt_d, bq_sb, qt_sb)
            vws, vas = load_wa(wv_d, vt_d)

            # --- attention + output projection, per query block
            # OP of qb-1 is spread into qb's pair-0 kc loop (PE slack there);
            # PV matmuls trail the exp by 2 kc steps so the PE never waits.
            def emit_op(qb, ots_prev):
                ops = []
                for oc in range(8):
                    osl = slice(oc * 128, (oc + 1) * 128)
                    pso = pj.tile([128, 512], F32, tag="pj", name="pso")
                    for c in range(2):
                        nc.tensor.matmul(
                            pso[:], wo_sb[c][:, osl], ots_prev[c][:],
                            start=(c == 0), stop=(c == 1),
                        )
                    st = osg.tile([128, 512], F32, tag="st", name="st")
                    nc.vector.tensor_copy(st[:], pso[:])
                    nc.sync.dma_start(
                        out_d[oc][:, qb * 512:(qb + 1) * 512], st[:])
                    ops.append(None)

            ots_prev = None
            for qb in range(NQ):
                qsl = slice(qb * 512, (qb + 1) * 512)
                ots = [ot.tile([128, 512], DT, tag=f"c{c}", name=f"otc{c}")
                       for c in range(2)]
                for pair in range(2):
                    first_blk = (qb == 0 and pair == 0)
                    depth = NK if first_blk else 2
                    acc = [px.tile([65, 512], F32, tag="x", name="acc")
                           for _ in range(2)]
                    pend = []
                    op_iter = None
                    if pair == 0 and ots_prev is not None:
                        op_iter = iter(range(8))
                    for kc in range(NK):
                        if first_blk:
                            v_chain(vws, vas, kc)
                        ksl = slice(kc * 128, (kc + 1) * 128)
                        ps1 = p1.tile([128, 1024], F32, tag="s", name="s")
                        for hh in range(2):
                            psl = slice(hh * 64, (hh + 1) * 64)
                            nc.tensor.matmul(
                                ps1[:, hh * 512:(hh + 1) * 512],
                                kt_sb[pair][psl, ksl],
                                qt_sb[pair][psl, qsl],
                                start=True, stop=True,
                            )
                        prob = pr.tile([128, 1024], DT, tag="p", name="p")
                        nc.scalar.activation(
                            prob[:], ps1[:], AF.Exp, scale=0.125
                        )
                        pend.append((kc, prob))
                        if len(pend) > depth:
                            pkc, pprob = pend.pop(0)
                            for hh in range(2):
                                nc.tensor.matmul(
                                    acc[hh][:], v_sb[pkc][:, pair * 2 + hh, :],
                                    pprob[:, hh * 512:(hh + 1) * 512],
                                    start=(pkc == 0), stop=(pkc == NK - 1),
                                )
                        if op_iter is not None and kc % 2 == 1:
                            oc = next(op_iter, None)
                            if oc is not None:
                                osl = slice(oc * 128, (oc + 1) * 128)
                                pso = pj.tile([128, 512], F32, tag="pj",
                                              name="pso")
                                for c in range(2):
                                    nc.tensor.matmul(
                                        pso[:], wo_sb[c][:, osl],
                                        ots_prev[c][:],
                                        start=(c == 0), stop=(c == 1),
                                    )
                                st = osg.tile([128, 512], F32, tag="st",
                                              name="st")
                                nc.vector.tensor_copy(st[:], pso[:])
                                nc.sync.dma_start(
                                    out_d[oc][:, (qb - 1) * 512:qb * 512],
                                    st[:])
                    for pkc, pprob in pend:
                        for hh in range(2):
                            nc.tensor.matmul(
                                acc[hh][:], v_sb[pkc][:, pair * 2 + hh, :],
                                pprob[:, hh * 512:(hh + 1) * 512],
                                start=(pkc == 0), stop=(pkc == NK - 1),
                            )
                    # normalize off-bank: free both acc banks first
                    uns, dens = [], []
                    for hh in range(2):
                        un = sm.tile([64, 512], F32, tag=f"un{hh}",
                                     name=f"un{hh}")
                        nc.vector.tensor_copy(un[:], acc[hh][0:64, :])
                        den = sm.tile([1, 512], F32, tag=f"den{hh}",
                                      name=f"den{hh}")
                        nc.vector.tensor_copy(den[:], acc[hh][64:65, :])
                        uns.append(un)
                        dens.append(den)
                    for hh in range(2):
                        rec = sm.tile([1, 512], F32, tag="rec", name="rec")
                        nc.vector.reciprocal_approx_fast(rec[:], dens[hh][:])
                        rb = sm.tile([64, 512], F32, tag="rb", name="rb")
                        nc.gpsimd.partition_broadcast(rb[:], rec[:])
                        nc.vector.tensor_mul(
                            ots[pair][hh * 64:(hh + 1) * 64, :],
                            uns[hh][:], rb[:],
                        )
                ots_prev = ots
            emit_op(NQ - 1, ots_prev)

    nc.compile()
    return nc


def _get_nc():
    if "nc" not in _CACHE:
        _CACHE["nc"] = _build()
    return _CACHE["nc"]


def kernel(Q, K, V, Wq, bq, Wk, bk, Wv, bv, Wo, bo):
    nc = _get_nc()
    f = np.float32
    bf = np.float16
    in_maps = []
    for core in range(8):
        b, g = divmod(core, 4)
        sl = slice(g * OL, (g + 1) * OL)
        in_maps.append({
            "qt": np.ascontiguousarray(Q[b].T, dtype=bf),
            "kt": np.ascontiguousarray(K[b].T, dtype=bf),
            "vt": np.ascontiguousarray(V[b].T, dtype=bf),
            "wqt": np.ascontiguousarray(Wq[sl].T, dtype=bf),
            "wkt": np.ascontiguousarray(Wk[sl].T, dtype=bf),
            "wvt": np.ascontiguousarray(Wv[sl].T, dtype=bf),
            "bq2": np.ascontiguousarray(bq[sl].reshape(2, 128, 1), dtype=f),
            "bk2": np.ascontiguousarray(bk[sl].reshape(2, 128, 1), dtype=f),
            "bv1": np.ascontiguousarray(bv[sl].reshape(1, OL), dtype=bf),
            "wot": np.ascontiguousarray(Wo[:, sl].T, dtype=bf),
        })
    res = run_bass_kernel_spmd(nc, in_maps, core_ids=list(range(8)))
    out = np.empty((B, S, D), np.float32)
    for b in range(B):
        acc = res.results[b * 4 + 0]["out_t"].astype(np.float64)
        for g in range(1, 4):
            acc += res.results[b * 4 + g]["out_t"]
        out[b] = (acc.T + bo).astype(np.float32)
    return out


# revision 12
# speedup vs baseline: 1.1377x; 1.0265x over previous
"""Multi-head attention (B=2, S=2048, D=1024, H=16) on 8 Trainium2 NeuronCores.

Sharding: core c -> batch b = c // 4, head-group g = c % 4 (4 heads = 256 proj
dims per core). Each core computes its 4 heads' attention plus the matching
slice of the output projection; the host sums the 4 partial outputs per batch
and adds bo.

Device layouts (matmul operands float32r = fp32 bits at bf16 PE rate):
  qT/kT [o, s]   : proj from host-transposed Q/K (contraction on partitions)
  v     [s, o]   : natural layout + ones column per head (softmax denominator
                   rides along row 64 of the PV matmul output)
  scoresT [k, q] : head pairs row-packed on the PE (base_partition 0/64);
                   both halves of a [128,1024] PSUM tile -> one wide Exp
  outT  [d, q]   : unnormalized; moved off PSUM fast, normalized with
                   reciprocal_approx_fast + GpSimd partition_broadcast
  out_pT [o, q]  : local slice of x @ Wo.T; host transposes + sums + bias
"""

import ml_dtypes
import numpy as np

import concourse.bass as bass
import concourse.mybir as mybir
import concourse.tile as tile
from concourse import bacc
from concourse.bass_utils import run_bass_kernel_spmd

B, S, D, H = 2, 2048, 1024, 16
OL = 256          # local projection dims (4 heads x 64)
NI = D // 128     # contraction chunks for projections
NK = S // 128     # key chunks
NQ = S // 512     # query blocks

_CACHE = {}


def _build():
    DT = mybir.dt.float16
    F32 = mybir.dt.float32
    AF = mybir.ActivationFunctionType

    nc = bacc.Bacc("TRN2", target_bir_lowering=False, debug=False, num_devices=8)

    qt_d = nc.dram_tensor("qt", [D, S], DT, kind="ExternalInput").ap() \
        .rearrange("(c p) s -> c p s", p=128)
    kt_d = nc.dram_tensor("kt", [D, S], DT, kind="ExternalInput").ap() \
        .rearrange("(c p) s -> c p s", p=128)
    vt_d = nc.dram_tensor("vt", [D, S], DT, kind="ExternalInput").ap() \
        .rearrange("(c p) s -> c p s", p=128)
    wqf_d = nc.dram_tensor("wqt", [D, OL], DT, kind="ExternalInput").ap() \
        .rearrange("(c p) o -> p c o", p=128)
    wkf_d = nc.dram_tensor("wkt", [D, OL], DT, kind="ExternalInput").ap() \
        .rearrange("(c p) o -> p c o", p=128)
    wvf_d = nc.dram_tensor("wvt", [D, OL], DT, kind="ExternalInput").ap() \
        .rearrange("(c p) o -> p c o", p=128)
    bq_d = nc.dram_tensor("bq2", [2, 128, 1], F32, kind="ExternalInput").ap()
    bk_d = nc.dram_tensor("bk2", [2, 128, 1], F32, kind="ExternalInput").ap()
    bv_d = nc.dram_tensor("bv1", [1, OL], DT, kind="ExternalInput").ap()
    wo_d = nc.dram_tensor("wot", [OL, D], DT, kind="ExternalInput").ap() \
        .rearrange("(c p) o -> c p o", p=128)
    out_d = nc.dram_tensor("out_t", [D, S], F32, kind="ExternalOutput").ap() \
        .rearrange("(c p) s -> c p s", p=128)

    with tile.TileContext(nc) as tc:
        with (
            tc.tile_pool(name="per", bufs=1) as per,
            tc.tile_pool(name="ip", bufs=1) as ip,
            tc.tile_pool(name="pr", bufs=8) as pr,
            tc.tile_pool(name="sm", bufs=2) as sm,
            tc.tile_pool(name="ot", bufs=2) as ot,
            tc.tile_pool(name="osg", bufs=3) as osg,
            tc.tile_pool(name="pj", bufs=2, space="PSUM") as pj,
            tc.tile_pool(name="p1", bufs=2, space="PSUM") as p1,
            tc.tile_pool(name="px", bufs=2, space="PSUM") as px,
        ):
            # --- persistent tiles
            qt_sb = [per.tile([128, S], DT, tag=f"qt{m}", name=f"qt{m}")
                     for m in range(2)]
            kt_sb = [per.tile([128, S], DT, tag=f"kt{m}", name=f"kt{m}")
                     for m in range(2)]
            v_sb = [per.tile([128, 4, 65], DT, tag=f"v{sc}", name=f"v{sc}")
                    for sc in range(NK)]
            wo_sb = [per.tile([128, D], DT, tag=f"wo{c}", name=f"wo{c}")
                     for c in range(2)]
            bq_sb = [per.tile([128, 1], F32, tag=f"bq{m}", name=f"bq{m}")
                     for m in range(2)]
            bk_sb = [per.tile([128, 1], F32, tag=f"bk{m}", name=f"bk{m}")
                     for m in range(2)]
            bv_sb = per.tile([1, OL], DT, tag="bv", name="bv")
            ones_f = per.tile([1, 128], F32, tag="ones_f", name="ones_f")
            vones_f = per.tile([128, 1], F32, tag="vones_f", name="vones_f")
            ones_r = per.tile([1, 128], DT, tag="ones_r", name="ones_r")
            nc.vector.memset(ones_f[:], 1.0)
            nc.vector.memset(vones_f[:], 1.0)
            nc.vector.tensor_copy(ones_r[:], ones_f[:])

            for m in range(2):
                nc.sync.dma_start(bq_sb[m][:], bq_d[m])
                nc.sync.dma_start(bk_sb[m][:], bk_d[m])
            nc.sync.dma_start(bv_sb[:], bv_d)
            for c in range(2):
                nc.sync.dma_start(wo_sb[c][:], wo_d[c])

            # --- input loads: K/Q stream via Sync HWDGE, V via GpSimd SWDGE
            def load_wa(w_flat, a_dr, key, eng):
                # one DMA for the whole weight tensor [1024, OL] -> [128,8,OL]
                wt = per.tile([128, NI, OL], DT, tag=f"wt{key}",
                              name=f"wt{key}")
                eng.dma_start(wt[:], w_flat)
                as_ = []
                for i in range(NI):
                    a = ip.tile([128, S], DT, tag=f"a{key}{i}",
                                name=f"a{key}{i}")
                    eng.dma_start(a[:], a_dr[i])
                    as_.append(a)
                return wt, as_

            wt_k, as_k = load_wa(wkf_d, kt_d, "k", nc.sync)
            wt_q, as_q = load_wa(wqf_d, qt_d, "q", nc.sync)
            wt_v, as_v = load_wa(wvf_d, vt_d, "v", nc.gpsimd)

            def proj_qk(wt, as_, bias_sb, dst_sb):
                # dst[o, s] = sum_i W[o, i] X[s, i]
                for s in range(4):
                    for m in range(2):
                        acc = pj.tile([128, 512], F32, tag="pj", name="pj")
                        for i in range(NI):
                            nc.tensor.matmul(
                                acc[:],
                                wt[:, i, m * 128:(m + 1) * 128],
                                as_[i][:, s * 512:(s + 1) * 512],
                                start=(i == 0),
                                stop=(i == NI - 1),
                            )
                        nc.scalar.activation(
                            dst_sb[m][:, s * 512:(s + 1) * 512],
                            acc[:], AF.Identity, bias=bias_sb[m][:],
                        )

            def v_chain(sc):
                # one v[s, o] output chunk: 8-deep accumulation + bias matmul
                acc = pj.tile([128, OL], F32, tag="pj", name="pj")
                for i in range(NI):
                    nc.tensor.matmul(
                        acc[:],
                        as_v[i][:, sc * 128:(sc + 1) * 128],
                        wt_v[:, i, :],
                        start=(i == 0),
                        stop=False,
                    )
                nc.tensor.matmul(
                    acc[:], ones_r[:], bv_sb[:], start=False, stop=True
                )
                for h in range(4):
                    nc.vector.tensor_copy(
                        v_sb[sc][:, h, 0:64],
                        acc[:, h * 64:(h + 1) * 64],
                    )

            for sc in range(NK):
                nc.vector.tensor_copy(
                    v_sb[sc][:, :, 64:65],
                    vones_f[:].to_broadcast((128, 4, 1)),
                )
            proj_qk(wt_k, as_k, bk_sb, kt_sb)
            for sc in range(NK):
                v_chain(sc)
            proj_qk(wt_q, as_q, bq_sb, qt_sb)

            # --- attention + output projection, per query block
            # OP of qb-1 is spread into qb's pair-0 kc loop (PE slack there);
            # PV matmuls trail the exp by 2 kc steps so the PE never waits.
            def emit_op(qb, ots_prev, pools):
                for oc in range(8):
                    osl = slice(oc * 128, (oc + 1) * 128)
                    pool, tg = pools[oc % len(pools)]
                    pso = pool.tile([128, 512], F32, tag=tg, name="pso")
                    for c in range(2):
                        nc.tensor.matmul(
                            pso[:], wo_sb[c][:, osl], ots_prev[c][:],
                            start=(c == 0), stop=(c == 1),
                        )
                    st = osg.tile([128, 512], F32, tag="st", name="st")
                    nc.vector.tensor_copy(st[:], pso[:])
                    nc.sync.dma_start(
                        out_d[oc][:, qb * 512:(qb + 1) * 512], st[:])

            ots_prev = None
            for qb in range(NQ):
                qsl = slice(qb * 512, (qb + 1) * 512)
                ots = [ot.tile([128, 512], DT, tag=f"c{c}", name=f"otc{c}")
                       for c in range(2)]
                for pair in range(2):
                    acc = [px.tile([65, 512], F32, tag="x", name="acc")
                           for _ in range(2)]
                    pend = []
                    op_iter = None
                    if pair == 0 and ots_prev is not None:
                        op_iter = iter(range(8))
                    for kc in range(NK):
                        ksl = slice(kc * 128, (kc + 1) * 128)
                        ps1 = p1.tile([128, 1024], F32, tag="s", name="s")
                        for hh in range(2):
                            psl = slice(hh * 64, (hh + 1) * 64)
                            nc.tensor.matmul(
                                ps1[:, hh * 512:(hh + 1) * 512],
                                kt_sb[pair][psl, ksl],
                                qt_sb[pair][psl, qsl],
                                start=True, stop=True,
                            )
                        prob = pr.tile([128, 1024], DT, tag="p", name="p")
                        nc.scalar.activation(
                            prob[:], ps1[:], AF.Exp, scale=0.125
                        )
                        pend.append((kc, prob))
                        if len(pend) > 2:
                            pkc, pprob = pend.pop(0)
                            for hh in range(2):
                                nc.tensor.matmul(
                                    acc[hh][:], v_sb[pkc][:, pair * 2 + hh, :],
                                    pprob[:, hh * 512:(hh + 1) * 512],
                                    start=(pkc == 0), stop=(pkc == NK - 1),
                                )
                        if op_iter is not None and kc % 2 == 1:
                            oc = next(op_iter, None)
                            if oc is not None:
                                osl = slice(oc * 128, (oc + 1) * 128)
                                pso = pj.tile([128, 512], F32, tag="pj",
                                              name="pso")
                                for c in range(2):
                                    nc.tensor.matmul(
                                        pso[:], wo_sb[c][:, osl],
                                        ots_prev[c][:],
                                        start=(c == 0), stop=(c == 1),
                                    )
                                st = osg.tile([128, 512], F32, tag="st",
                                              name="st")
                                nc.vector.tensor_copy(st[:], pso[:])
                                nc.sync.dma_start(
                                    out_d[oc][:, (qb - 1) * 512:qb * 512],
                                    st[:])
                    for pkc, pprob in pend:
                        for hh in range(2):
                            nc.tensor.matmul(
                                acc[hh][:], v_sb[pkc][:, pair * 2 + hh, :],
                                pprob[:, hh * 512:(hh + 1) * 512],
                                start=(pkc == 0), stop=(pkc == NK - 1),
                            )
                    # normalize off-bank: free both acc banks first
                    uns, dens = [], []
                    for hh in range(2):
                        un = sm.tile([64, 512], F32, tag=f"un{hh}",
                                     name=f"un{hh}")
                        nc.vector.tensor_copy(un[:], acc[hh][0:64, :])
                        den = sm.tile([1, 512], F32, tag=f"den{hh}",
                                      name=f"den{hh}")
                        nc.vector.tensor_copy(den[:], acc[hh][64:65, :])
                        uns.append(un)
                        dens.append(den)
                    for hh in range(2):
                        rec = sm.tile([1, 512], F32, tag="rec", name="rec")
                        nc.vector.reciprocal_approx_fast(rec[:], dens[hh][:])
                        rb = sm.tile([64, 512], F32, tag="rb", name="rb")
                        nc.gpsimd.partition_broadcast(rb[:], rec[:])
                        nc.vector.tensor_mul(
                            ots[pair][hh * 64:(hh + 1) * 64, :],
                            uns[hh][:], rb[:],
                        )
                ots_prev = ots
            emit_op(NQ - 1, ots_prev, [(pj, "pj"), (px, "x")])

    nc.compile()
    return nc


def _get_nc():
    if "nc" not in _CACHE:
        _CACHE["nc"] = _build()
    return _CACHE["nc"]


def kernel(Q, K, V, Wq, bq, Wk, bk, Wv, bv, Wo, bo):
    nc = _get_nc()
    f = np.float32
    bf = np.float16
    in_maps = []
    for core in range(8):
        b, g = divmod(core, 4)
        sl = slice(g * OL, (g + 1) * OL)
        in_maps.append({
            "qt": np.ascontiguousarray(Q[b].T, dtype=bf),
            "kt": np.ascontiguousarray(K[b].T, dtype=bf),
            "vt": np.ascontiguousarray(V[b].T, dtype=bf),
            "wqt": np.ascontiguousarray(Wq[sl].T, dtype=bf),
            "wkt": np.ascontiguousarray(Wk[sl].T, dtype=bf),
            "wvt": np.ascontiguousarray(Wv[sl].T, dtype=bf),
            "bq2": np.ascontiguousarray(bq[sl].reshape(2, 128, 1), dtype=f),
            "bk2": np.ascontiguousarray(bk[sl].reshape(2, 128, 1), dtype=f),
            "bv1": np.ascontiguousarray(bv[sl].reshape(1, OL), dtype=bf),
            "wot": np.ascontiguousarray(Wo[:, sl].T, dtype=bf),
        })
    res = run_bass_kernel_spmd(nc, in_maps, core_ids=list(range(8)))
    out = np.empty((B, S, D), np.float32)
    for b in range(B):
        acc = res.results[b * 4 + 0]["out_t"].astype(np.float64)
        for g in range(1, 4):
            acc += res.results[b * 4 + g]["out_t"]
        out[b] = (acc.T + bo).astype(np.float32)
    return out
